# revision 14
# baseline (speedup 1.0000x reference)
"""Trainium2 Bass kernel for the per-cell-MLP "MAR one-sided missingness" model.

Model (per cell (n,t) of a 1024x128 grid):
    xc     = X[n, col_idx[n,t]]
    h      = relu(W_in[n,t,:,0]*xc + W_in[n,t,:,1]*X[n,t] + b_in[n,t,:])   # [H]
    out    = sigmoid(dot(W_out[n,t,:], h) + b_out[n,t])

Sharding: rows N split across 8 cores (128 rows each), fully data parallel.

Per-core layout: partition dim = t (128 cells of one row), free dim = h.
The neighbor gather X[n, col_idx[n,t]] runs on the PE as a one-hot matmul:
one-hot masks (a host-staged re-encoding of col_idx) are fp8 stationaries;
X rides as an f16 hi/lo split (lo pre-scaled by 2^12 to avoid denormals) so
the gathered values are exact to ~2.5e-7 relative.

Weights stream as four contiguous [t, n, h] tensors (w0, w1, b, wo).
Per superblock of G rows (variable G: small blocks at the start/end to
shorten pipeline ramp and tail):
  PE   : xc2[:, 2g:2g+2] = oh_g^T @ [Xhi | Xlo*2^12][:, n]   (per row)
  ACT  : xc2_sb copy; xc = (lo * 2^-12) + hi   (DVE, batched)
  ACT  : a0_g = w0_g * xc_g                    (per row, per-partition scale)
  DVE  : m1   = w1 * broadcast(x)              (batched TT, stride-0 AP)
  Pool : v    = m1 + b                         (batched)
  Pool : u    = a0 + v                         (batched)
  DVE  : r    = (u max 0) * wo                 (batched STT)
  DVE  : red[:, g] = sum_h r                   (batched reduce)
Epilogue: out = sigmoid(red + b_out^T), DMA out, host transposes back.

HBM-bandwidth bound: streams ~34 MB per core.
"""

import ml_dtypes
import numpy as np

N, T, H = 1024, 128, 128
M = 8            # cores
NR = N // M      # rows per core
SIZES = [4, 4, 8] + [16] * 6 + [8, 4, 4]   # sums to 128
assert sum(SIZES) == NR
GMAX = max(SIZES)
LO_SCALE = float(2 ** 12)

_cache = {}


def _build():
    if "nc" in _cache:
        return _cache["nc"]
    import concourse.bacc as bacc
    import concourse.mybir as mybir
    import concourse.tile as tile

    f32 = mybir.dt.float32
    f16 = mybir.dt.float16
    f8 = mybir.dt.float8e4
    Alu = mybir.AluOpType
    Act = mybir.ActivationFunctionType

    nc = bacc.Bacc()
    w0all = nc.declare_dram_parameter("w0all", [T, NR, H], f32, isOutput=False)
    w1all = nc.declare_dram_parameter("w1all", [T, NR, H], f32, isOutput=False)
    ball = nc.declare_dram_parameter("ball", [T, NR, H], f32, isOutput=False)
    woall = nc.declare_dram_parameter("woall", [T, NR, H], f32, isOutput=False)
    ohall = nc.declare_dram_parameter("ohall", [128, NR * T], f8, isOutput=False)
    xt = nc.declare_dram_parameter("xt", [T, NR], f32, isOutput=False)
    xhl = nc.declare_dram_parameter("xhl", [128, NR, 2], f16, isOutput=False)
    bout = nc.declare_dram_parameter("bout", [T, NR], f32, isOutput=False)
    out = nc.declare_dram_parameter("out", [T, NR], f32, isOutput=True)

    with tile.TileContext(nc) as tc:
        with (
            tc.tile_pool(name="const", bufs=1) as constp,
            tc.tile_pool(name="wpool", bufs=2) as wpool,
            tc.tile_pool(name="ohp", bufs=2) as ohp,
            tc.tile_pool(name="work", bufs=2) as workp,
            tc.tile_pool(name="acc", bufs=1) as accp,
            tc.tile_pool(name="psxc", bufs=2, space="PSUM") as psxcp,
        ):
            xt_sb = constp.tile([T, NR], f32)
            nc.scalar.dma_start(xt_sb[:], xt[:])
            xhl_sb = constp.tile([128, NR * 2], f16)
            nc.scalar.dma_start(xhl_sb[:], xhl[:])
            bo_sb = constp.tile([T, NR], f32)
            nc.scalar.dma_start(bo_sb[:], bout[:])

            red = accp.tile([T, NR], f32)

            n0 = 0
            for s, G in enumerate(SIZES):
                nsl = slice(n0, n0 + G)
                w0a = wpool.tile([128, G * H], f32, tag="w0a")
                nc.sync.dma_start(w0a[:], w0all[:, nsl])
                w1a = wpool.tile([128, G * H], f32, tag="w1a")
                nc.sync.dma_start(w1a[:], w1all[:, nsl])
                ba = wpool.tile([128, G * H], f32, tag="ba")
                nc.sync.dma_start(ba[:], ball[:, nsl])
                woa = wpool.tile([128, G * H], f32, tag="woa")
                nc.sync.dma_start(woa[:], woall[:, nsl])
                oh = ohp.tile([128, G * T], f8, tag="oh")
                nc.scalar.dma_start(
                    oh[:], ohall[:, n0 * T : (n0 + G) * T]
                )

                xc2_ps = psxcp.tile([128, 2 * G], f32, tag="xc")
                for g in range(G):
                    n = n0 + g
                    nc.tensor.matmul(
                        xc2_ps[:, 2 * g : 2 * g + 2],
                        oh[:, g * T : (g + 1) * T],
                        xhl_sb[:, 2 * n : 2 * n + 2],
                        start=True,
                        stop=True,
                    )
                xc2_sb = workp.tile([128, 2 * G], f32, tag="xc2sb")
                nc.scalar.copy(xc2_sb[:], xc2_ps[:])
                xc_sb = workp.tile([128, G], f32, tag="xcsb")
                nc.vector.scalar_tensor_tensor(
                    xc_sb[:],
                    xc2_sb[:].rearrange("p (g k) -> p g k", k=2)[:, :, 1],
                    1.0 / LO_SCALE,
                    xc2_sb[:].rearrange("p (g k) -> p g k", k=2)[:, :, 0],
                    Alu.mult,
                    Alu.add,
                )

                m1 = workp.tile([128, G * H], f32, tag="m1")
                nc.vector.tensor_tensor(
                    m1[:].rearrange("p (g h) -> p g h", g=G),
                    w1a[:].rearrange("p (g h) -> p g h", g=G),
                    xt_sb[:, nsl].broadcast_to([128, G, H]),
                    Alu.mult,
                )
                v = workp.tile([128, G * H], f32, tag="v")
                nc.gpsimd.tensor_tensor(v[:], m1[:], ba[:], Alu.add)

                a0 = workp.tile([128, G * H], f32, tag="a0")
                for g in range(G):
                    nc.scalar.activation(
                        a0[:, g * H : (g + 1) * H],
                        w0a[:, g * H : (g + 1) * H],
                        Act.Copy,
                        scale=xc_sb[:, g : g + 1],
                    )
                u = workp.tile([128, G * H], f32, tag="u")
                nc.gpsimd.tensor_tensor(u[:], a0[:], v[:], Alu.add)

                r = workp.tile([128, G * H], f32, tag="r")
                nc.vector.scalar_tensor_tensor(
                    r[:], u[:], 0.0, woa[:], Alu.max, Alu.mult
                )
                nc.vector.tensor_reduce(
                    red[:, nsl],
                    r[:].rearrange("p (g h) -> p g h", g=G),
                    axis=mybir.AxisListType.X,
                    op=Alu.add,
                )
                n0 += G

            lg = workp.tile([T, NR], f32, tag="lg")
            nc.vector.tensor_tensor(lg[:], red[:], bo_sb[:], Alu.add)
            ot = workp.tile([T, NR], f32, tag="ot")
            nc.scalar.activation(ot[:], lg[:], Act.Sigmoid)
            nc.sync.dma_start(out[:], ot[:])

    nc.compile()
    _cache["nc"] = nc
    return nc


def make_in_maps(X, W_in, b_in, W_out, b_out, col_idx):
    X = np.asarray(X, dtype=np.float32)
    W_in = np.asarray(W_in, dtype=np.float32)
    b_in = np.asarray(b_in, dtype=np.float32)
    W_out = np.asarray(W_out, dtype=np.float32)
    b_out = np.asarray(b_out, dtype=np.float32)
    col_idx = np.asarray(col_idx)

    jj = np.arange(128)
    in_maps = []
    for c in range(M):
        sl = slice(c * NR, (c + 1) * NR)
        Wc = W_in[sl]  # [NR, T, H, 2]
        w0all = np.ascontiguousarray(Wc[:, :, :, 0].transpose(1, 0, 2))
        w1all = np.ascontiguousarray(Wc[:, :, :, 1].transpose(1, 0, 2))
        ball = np.ascontiguousarray(b_in[sl].transpose(1, 0, 2))
        woall = np.ascontiguousarray(W_out[sl].transpose(1, 0, 2))

        ohall = (col_idx[sl].reshape(1, -1) == jj[:, None]).astype(
            ml_dtypes.float8_e4m3
        )

        xtc = np.ascontiguousarray(X[sl].T)  # [t, n] f32
        xhi = xtc.astype(np.float16)
        xlo = ((xtc - xhi.astype(np.float32)) * LO_SCALE).astype(np.float16)
        xhl = np.stack([xhi, xlo], axis=-1)  # [128, NR, 2]

        in_maps.append(
            {
                "w0all": w0all,
                "w1all": w1all,
                "ball": ball,
                "woall": woall,
                "ohall": ohall,
                "xt": xtc,
                "xhl": xhl,
                "bout": np.ascontiguousarray(b_out[sl].T),
            }
        )
    return in_maps


def kernel(X, W_in, b_in, W_out, b_out, col_idx):
    from concourse.bass_utils import run_bass_kernel_spmd

    nc = _build()
    in_maps = make_in_maps(X, W_in, b_in, W_out, b_out, col_idx)
    res = run_bass_kernel_spmd(nc, in_maps, list(range(M))).results
    out = np.empty((N, T), np.float32)
    for c in range(M):
        out[c * NR : (c + 1) * NR] = res[c]["out"].T
    return out
